# revision 29
# baseline (speedup 1.0000x reference)
"""CAGPool layer (score -> per-graph top-k -> gather/gate -> edge re-index)
as a Bass kernel on 8 Trainium2 NeuronCores.

Sharding: data-parallel by graph. Each of the 8 cores owns 32 consecutive
graphs (32*512 nodes, 32*16384 edges). No cross-core communication.

Per-core pipeline:
  1. scores d[i] = x[i] . pool_vector[graph(i)]  (DVE dot products, f32)
  2. per-graph ordered top-256 via DVE max8/max_index/match_replace rounds
  3. nodemap (local node id -> new global id + 1, 0 if dropped) via gpsimd
     local_scatter; affine fixup to (new id | -1) on DVE
  4. edge re-indexing: per-edge table lookups via gpsimd ap_gather
     (8 Q7 cores, one graph per core per round, 16-way wrapped lists),
     de-interleaved to edge order on the Scalar engine, masked on DVE
  5. x_out: 128-row indirect-DMA gathers (one offset per partition) from x,
     scaled by sigmoid(score/||p||) with per-partition scalars
"""

import numpy as np

NCORES = 8
B, N_PER, D, DEG = 256, 512, 256, 32
K = 256                      # kept nodes per graph (ceil(0.5 * 512))
GPC = B // NCORES            # 32 graphs per core
NPC = GPC * N_PER            # 16384 nodes per core
KPC = GPC * K                # 8192 kept nodes per core
EPG = N_PER * DEG            # 16384 edges per graph
EPC = GPC * EPG              # 524288 edges per core

NROUND = 4                   # graph rounds (8 graphs each, one per Q7 core)
NQ = 4                       # list quarters per round
LIDX = EPG // NQ             # 4096 indices per ap_gather call per core
LPP = LIDX // 16             # 256 list entries per partition

_cache = {}


def _build_nc():
    import concourse.bass as bass
    import concourse.bacc as bacc
    import concourse.mybir as mybir
    from concourse import tile
    from concourse.bass import IndirectOffsetOnAxis
    from concourse.masks import make_identity

    f32 = mybir.dt.float32
    i32 = mybir.dt.int32
    i16 = mybir.dt.int16
    u32 = mybir.dt.uint32
    u8 = mybir.dt.uint8
    Alu = mybir.AluOpType
    Act = mybir.ActivationFunctionType

    nc = bacc.Bacc("TRN2", target_bir_lowering=False, debug=False)

    x = nc.dram_tensor("x", [NPC, D], f32, kind="ExternalInput")
    ei = nc.dram_tensor("edge_index", [2, EPC], i32, kind="ExternalInput")
    pv = nc.dram_tensor("pool_vector", [GPC, D], f32, kind="ExternalInput")
    # per-core f32 constants, per partition row p:
    #   col0: core*NPC                col1: p*N_PER
    #   col2: core*KPC + p*K          col3: core*GPC + p
    #   col4: core*NPC + p*N_PER      col5+R: (core*GPC + R*8 + p//16)*N_PER
    consts = nc.dram_tensor("consts", [128, 44], f32, kind="ExternalInput")
    # column-major edge aux for the DVE compare-gather channel:
    # ei_cols[side, G*EPG + q*LIDX + p*32 + c] = ei[side, G*EPG + q*LIDX + c*128 + p]
    eic = nc.dram_tensor("ei_cols", [2, EPC], i32, kind="ExternalInput")

    x_out = nc.dram_tensor("x_out", [KPC, D], f32, kind="ExternalOutput")
    e_new = nc.dram_tensor("edge_new", [2, EPC], i32, kind="ExternalOutput")
    batch_out = nc.dram_tensor("batch_out", [KPC], i32, kind="ExternalOutput")
    perm_out = nc.dram_tensor("perm", [KPC], i32, kind="ExternalOutput")
    valid_out = nc.dram_tensor("valid", [EPC], u8, kind="ExternalOutput")

    with tile.TileContext(nc) as tc:
        with (
            tc.tile_pool(name="persist", bufs=1) as pp,
            tc.tile_pool(name="xload", bufs=3) as xp,
            tc.tile_pool(name="edge", bufs=2) as ep,
            tc.tile_pool(name="tabs", bufs=4) as tbp,
            tc.tile_pool(name="og", bufs=1) as ogp,
            tc.tile_pool(name="l16s", bufs=24) as lp,
            tc.tile_pool(name="evs", bufs=6) as vp,
            tc.tile_pool(name="xg", bufs=8) as gp,
            tc.tile_pool(name="psum", bufs=2, space="PSUM") as psp,
            tc.tile_pool(name="dram", bufs=1, space="DRAM") as dp,
        ):
            # ---- persistent small tiles ----
            consts_sb = pp.tile([128, 44], f32, tag="consts")
            nc.sync.dma_start(consts_sb[:, :], consts[:, :])
            pv_sb = pp.tile([GPC, D], f32, tag="pv_sb")
            nc.sync.dma_start(pv_sb[:, :], pv[:, :])
            ident = pp.tile([128, 128], f32, tag="ident")
            make_identity(nc, ident[:, :])

            gnode_off = consts_sb[0:GPC, 1:2]       # [32,1] p*N_PER
            gnew_off = consts_sb[0:GPC, 2:3]        # [32,1] core*KPC + p*K
            g_id = consts_sb[0:GPC, 3:4]            # [32,1] core*GPC + p
            gnode_glob = consts_sb[0:GPC, 4:5]      # [32,1] core*NPC + p*N_PER

            # ---- phase 1: scores (d = x . p_g) ----
            sc_raw = pp.tile([128, 128], f32, tag="sc_raw")
            scorep = tc.tile_pool(name="scorep", bufs=1)
            scp = scorep.__enter__()
            xloadp = tc.tile_pool(name="xload2", bufs=3)
            xp2 = xloadp.__enter__()
            pv_bc = scp.tile([128, GPC * D], f32, tag="pv_bc")
            nc.sync.dma_start(
                pv_bc[:, :],
                pv[:, :].rearrange("a b -> (a b)").partition_broadcast(128),
            )
            junk = scp.tile([128, D], f32, tag="junk")
            TGROUP = 8
            for tg in range(128 // TGROUP):
                xt = xp2.tile([128, TGROUP * D], f32, tag="xt")
                nc.sync.dma_start(
                    xt[:, :].rearrange("p (t d) -> p t d", t=TGROUP),
                    x[tg * TGROUP * 128:(tg + 1) * TGROUP * 128, :].rearrange(
                        "(t p) d -> p t d", p=128
                    ),
                )
                for t_in in range(TGROUP):
                    t = tg * TGROUP + t_in
                    g = t // 4
                    nc.vector.scalar_tensor_tensor(
                        out=junk[:, :],
                        in0=xt[:, t_in * D:(t_in + 1) * D],
                        scalar=0.0,
                        in1=pv_bc[:, g * D:(g + 1) * D],
                        op0=Alu.bypass,
                        op1=Alu.mult,
                        accum_out=sc_raw[:, t:t + 1],
                    )

            # ---- phase 2: transpose scores to [graph, node] layout ----
            psT = psp.tile([128, 128], f32, tag="psT")
            nc.tensor.transpose(psT[:, :], sc_raw[:, :], ident[:, :])
            scT = pp.tile([128, 128], f32, tag="scT")
            nc.scalar.copy(scT[:, :], psT[:, :])
            sc_rt = dp.tile([128, 128], f32, tag="sc_rt")
            nc.sync.dma_start(sc_rt[:, :], scT[:, :])
            work = pp.tile([GPC, N_PER], f32, tag="work")
            nc.sync.dma_start(
                work[:, :],
                sc_rt[:, :].rearrange("a b -> (a b)").rearrange("(g j) -> g j", g=GPC),
            )
            xloadp.__exit__(None, None, None)
            scorep.__exit__(None, None, None)

            # ---- phase 3: ordered top-256 per graph ----
            vals = pp.tile([GPC, K], f32, tag="vals")
            idxs = pp.tile([GPC, K], u32, tag="idxs")
            NR = K // 8
            for r in range(NR):
                sl = slice(8 * r, 8 * (r + 1))
                nc.vector.max(vals[:, sl], work[:, :])
                nc.vector.max_index(idxs[:, sl], vals[:, sl], work[:, :])
                if r != NR - 1:
                    nc.vector.match_replace(
                        work[:, :], vals[:, sl], work[:, :], imm_value=-1e30
                    )

            # ---- phase 4: gate + perm + batch ----
            junk2 = pp.tile([GPC, D], f32, tag="junk2")
            nrm2 = pp.tile([GPC, 1], f32, tag="nrm2")
            nc.vector.scalar_tensor_tensor(
                out=junk2[:, :], in0=pv_sb[:, :], scalar=0.0, in1=pv_sb[:, :],
                op0=Alu.bypass, op1=Alu.mult, accum_out=nrm2[:, :],
            )
            nrm = pp.tile([GPC, 1], f32, tag="nrm")
            nc.scalar.activation(nrm[:, :], nrm2[:, :], Act.Sqrt)
            invn = pp.tile([GPC, 1], f32, tag="invn")
            nc.vector.reciprocal(invn[:, :], nrm[:, :])
            gate = pp.tile([GPC, K], f32, tag="gate")
            nc.scalar.activation(gate[:, :], vals[:, :], Act.Sigmoid, scale=invn[:, :])

            idx32 = pp.tile([GPC, K], i32, tag="idx32")
            nc.vector.tensor_copy(idx32[:, :], idxs[:, :])
            perm_t = pp.tile([GPC, K], i32, tag="perm_t")
            nc.vector.tensor_scalar(
                perm_t[:, :], idx32[:, :], gnode_glob, None, op0=Alu.add
            )
            nc.sync.dma_start(
                perm_out[:].rearrange("(g r) -> g r", g=GPC), perm_t[:, :]
            )
            batch_t = pp.tile([GPC, K], i32, tag="batch_t")
            nc.vector.memset(batch_t[:, :], 0)
            nc.vector.tensor_scalar(
                batch_t[:, :], batch_t[:, :], g_id, None, op0=Alu.add
            )
            nc.sync.dma_start(
                batch_out[:].rearrange("(g r) -> g r", g=GPC), batch_t[:, :]
            )

            # ---- phase 4b: j-ordered [128, 64] layouts of row ids and gates
            # (OFF[p, c] = local row id of x_out row j = c*128 + p; GT same for
            # gate).  j = g*256 + r -> p = r % 128, c = 2g + r//128, so OFF is
            # the PE transpose of idx rows split in two 128-halves.
            idxF = pp.tile([GPC, K], f32, tag="idxF")
            nc.vector.tensor_copy(idxF[:, :], idx32[:, :])
            nc.vector.tensor_scalar(
                idxF[:, :], idxF[:, :], gnode_off, None, op0=Alu.add
            )
            OFFf = pp.tile([128, 2 * GPC], f32, tag="OFFf")
            GTt = pp.tile([128, 2 * GPC], f32, tag="GTt")
            for h in range(2):
                psA = psp.tile([128, GPC], f32, tag="psA")
                nc.tensor.transpose(
                    psA[:, :], idxF[:, h * 128:(h + 1) * 128], ident[0:GPC, 0:GPC]
                )
                nc.scalar.copy(
                    OFFf[:, :].rearrange("p (c h2) -> p c h2", h2=2)[:, :, h],
                    psA[:, :],
                )
                psB = psp.tile([128, GPC], f32, tag="psB")
                nc.tensor.transpose(
                    psB[:, :], gate[:, h * 128:(h + 1) * 128], ident[0:GPC, 0:GPC]
                )
                nc.scalar.copy(
                    GTt[:, :].rearrange("p (c h2) -> p c h2", h2=2)[:, :, h],
                    psB[:, :],
                )
            OFF = pp.tile([128, 2 * GPC], i32, tag="OFF")
            nc.vector.tensor_copy(OFF[:, :], OFFf[:, :])

            # ---- phase 7: x_out row gather + sigmoid gating ----
            for c in range(KPC // 128):
                xg = gp.tile([128, D], f32, tag="xgt")
                nc.gpsimd.indirect_dma_start(
                    xg[:, :], None, x[:, :],
                    IndirectOffsetOnAxis(ap=OFF[:, c:c + 1], axis=0),
                )
                nc.vector.tensor_scalar(
                    xg[:, :], xg[:, :], GTt[:, c:c + 1], None, op0=Alu.mult
                )
                nc.sync.dma_start(
                    x_out[c * 128:(c + 1) * 128, :], xg[:, :]
                )


            # ---- phase 5: nodemap table (v+1 scheme, 0 = dropped) ----
            # nm[g, v] = sum_r [idx[g, r] == v] * (r+1), built with 512
            # compare-accumulate STT ops (one per node slot v); runs on DVE
            # and hides completely under the pool-bound edge phase.
            idxsF = pp.tile([GPC, K], f32, tag="idxsF")
            nc.vector.tensor_copy(idxsF[:, :], idxs[:, :])
            rr_i = pp.tile([GPC, K], i32, tag="rr_i")
            nc.gpsimd.iota(rr_i[:, :], pattern=[[1, K]], base=1, channel_multiplier=0)
            rr_bc = pp.tile([GPC, K], f32, tag="rr_bc")
            nc.vector.tensor_copy(rr_bc[:, :], rr_i[:, :])
            junk3 = pp.tile([GPC, K], f32, tag="junk3")
            nmF = pp.tile([GPC, N_PER], f32, tag="nmF")
            for v in range(N_PER):
                nc.vector.scalar_tensor_tensor(
                    out=junk3[:, :], in0=idxsF[:, :], scalar=float(v),
                    in1=rr_bc[:, :], op0=Alu.is_equal, op1=Alu.mult,
                    accum_out=nmF[:, v:v + 1],
                )
            nm32 = pp.tile([GPC, N_PER], i32, tag="nm32")
            nc.vector.tensor_copy(nm32[:, :], nmF[:, :])
            nmask = pp.tile([GPC, N_PER], i32, tag="nmask")
            nc.vector.tensor_scalar(nmask[:, :], nm32[:, :], 0, None, op0=Alu.is_gt)
            nm_t1 = pp.tile([GPC, N_PER], i32, tag="nm_t1")
            nc.vector.tensor_scalar(
                nm_t1[:, :], nm32[:, :], gnew_off, None, op0=Alu.add
            )
            nm_t2 = pp.tile([GPC, N_PER], i32, tag="nm_t2")
            nc.vector.tensor_tensor(nm_t2[:, :], nm_t1[:, :], nmask[:, :], op=Alu.mult)
            nm = pp.tile([GPC, N_PER], i32, tag="nm")
            nc.vector.tensor_scalar(nm[:, :], nm_t2[:, :], 1, None, op0=Alu.subtract)
            nm_dram = dp.tile([GPC, N_PER], i32, tag="nm_dram")
            nc.sync.dma_start(nm_dram[:, :], nm[:, :])

            # ---- phase 6: edge re-indexing via ap_gather rounds ----
            # Round R: Q7 core j handles graph G = R*8 + j; its 16 partitions
            # hold graph G's nodemap (replicated).  The host passes the edge
            # array 16-way pre-interleaved per 4096-edge quarter, so the
            # wrapped per-core index list unwraps to plain edge order: the
            # gather output og[p, k] = nodemap[src[e = Qq*4096 + k]].  Each
            # partition then keeps its own 256-edge slice via one diagonal
            # SBUF->SBUF DMA and the mask/write phase runs on dense tiles.
            # 6a: all index lists first (keeps the in-order DVE stream from
            # blocking the pool's gathers behind later mask work)
            DVE_PAIRS_EARLY = [(R, 3) for R in range(NROUND)] + [(0, 2), (1, 2)]
            iters = [(R, Qq, side)
                     for R in range(NROUND) for Qq in range(NQ) for side in range(2)
                     if (R, Qq) not in DVE_PAIRS_EARLY]
            l16s = {}
            for R, Qq, side in iters:
                eb = ep.tile([128, LPP], i32, tag="eb")
                nc.sync.dma_start(
                    eb[:, :],
                    ei[side, :]
                    .rearrange("(g e) -> g e", g=GPC)[R * 8:(R + 1) * 8, :]
                    .rearrange("g (q w s) -> g q w s", q=NQ, w=16)
                    [:, Qq, :, :],
                )
                loc = ep.tile([128, LPP], i32, tag="loc")
                nc.vector.tensor_scalar(
                    loc[:, :], eb[:, :], consts_sb[:, 5 + R:6 + R], None,
                    op0=Alu.subtract,
                )
                l16 = lp.tile([128, LPP], i16, tag="l16")
                nc.vector.tensor_copy(l16[:, :], loc[:, :])
                l16s[(R, Qq, side)] = l16

            DVE_PAIRS = [(R, 3) for R in range(NROUND)] + [(0, 2), (1, 2)]
            # 6b: per (R, Qq): gathers + bounce extraction + masks + writes.
            # All DVE index prep already queued (6a), so the in-order DVE
            # stream can park on masks without starving the pool.
            # ---- 6d: DVE compare-gather channel for DVE_PAIRS quarters ----
            # Gathers nodemap[src] as sum_v [v == src] * nmf[v] with one
            # scalar_tensor_tensor per 128-edge column (5.4 ns/edge on DVE),
            # running concurrently with the pool-bound ap_gather channel.
            iotaI = pp.tile([128, N_PER], i32, tag="iotaI")
            nc.gpsimd.iota(
                iotaI[:, :], pattern=[[1, N_PER]], base=0, channel_multiplier=0
            )
            iotaF = pp.tile([128, N_PER], f32, tag="iotaF")
            nc.vector.tensor_copy(iotaF[:, :], iotaI[:, :])
            nmFf = pp.tile([GPC, N_PER], f32, tag="nmFf")
            nc.vector.tensor_copy(nmFf[:, :], nm[:, :])
            nmf_dram = dp.tile([GPC, N_PER], f32, tag="nmf_dram")
            nc.sync.dma_start(nmf_dram[:, :], nmFf[:, :])
            nmf_rows = nmf_dram[:, :].rearrange("a b -> (a b)").rearrange(
                "(g j) -> g j", g=GPC
            )
            junkc = pp.tile([128, N_PER], f32, tag="junkc")
            for R, Qq in DVE_PAIRS:
                rvT = {}
                for side_ in range(2):
                    for gq_ in range(2):
                        rvT_tile = ep.tile(
                            [128, 128], f32, tag=f"rvT{side_}{gq_}"
                        )
                        rvT[(side_, gq_)] = rvT_tile
                for j in range(8):
                    G = R * 8 + j
                    nmb = ep.tile([128, N_PER], f32, tag="nmb")
                    nc.sync.dma_start(
                        nmb[:, :], nmf_rows[G, :].partition_broadcast(128)
                    )
                    for side in range(2):
                        scol = ep.tile([128, 32], i32, tag="scol")
                        nc.sync.dma_start(
                            scol[:, :],
                            eic[side, G * EPG + Qq * LIDX:
                                G * EPG + (Qq + 1) * LIDX]
                            .rearrange("(p c) -> p c", p=128),
                        )
                        lt = ep.tile([128, 32], i32, tag="lt")
                        nc.vector.tensor_scalar(
                            lt[:, :], scol[:, :], consts_sb[:, 12 + G:13 + G],
                            None, op0=Alu.subtract,
                        )
                        lf = ep.tile([128, 32], f32, tag="lf")
                        nc.vector.tensor_copy(lf[:, :], lt[:, :])
                        rvc = ep.tile([128, 32], f32, tag=f"rvc{side}")
                        for c in range(32):
                            nc.vector.scalar_tensor_tensor(
                                out=junkc[:, :], in0=iotaF[:, :],
                                scalar=lf[:, c:c + 1], in1=nmb[:, :],
                                op0=Alu.is_equal, op1=Alu.mult,
                                accum_out=rvc[:, c:c + 1],
                            )
                        psG = psp.tile([32, 128], f32, tag="psG")
                        nc.tensor.transpose(psG[:, :], rvc[:, :], ident[:, :])
                        nc.scalar.copy(
                            rvT[(side, j // 4)][:, :]
                            [32 * (j % 4):32 * (j % 4 + 1), :],
                            psG[:, :],
                        )
                for gq in range(2):
                    rv = rvT[(0, gq)]
                    cv = rvT[(1, gq)]
                    amF = ep.tile([128, 128], f32, tag="amF")
                    nc.vector.tensor_scalar(
                        amF[:, :], rv[:, :], 0, None, op0=Alu.is_ge
                    )
                    bmF = ep.tile([128, 128], f32, tag="bmF")
                    nc.vector.tensor_scalar(
                        bmF[:, :], cv[:, :], 0, None, op0=Alu.is_ge
                    )
                    valdF = ep.tile([128, 128], u8, tag="valdF")
                    nc.vector.tensor_tensor(
                        valdF[:, :], amF[:, :], bmF[:, :], op=Alu.mult
                    )
                    taF = ep.tile([128, 128], f32, tag="taF")
                    nc.vector.tensor_scalar(
                        taF[:, :], amF[:, :], (1 << 20), 1,
                        op0=Alu.mult, op1=Alu.subtract,
                    )
                    tbF = ep.tile([128, 128], f32, tag="tbF")
                    nc.vector.tensor_scalar(
                        tbF[:, :], bmF[:, :], (1 << 20), 1,
                        op0=Alu.mult, op1=Alu.subtract,
                    )
                    rwF = ep.tile([128, 128], f32, tag="rwF")
                    nc.vector.tensor_tensor(
                        rwF[:, :], rv[:, :], tbF[:, :], op=Alu.min
                    )
                    cwF = ep.tile([128, 128], f32, tag="cwF")
                    nc.vector.tensor_tensor(
                        cwF[:, :], cv[:, :], taF[:, :], op=Alu.min
                    )
                    rw32 = ep.tile([128, 128], i32, tag="rw32")
                    nc.vector.tensor_copy(rw32[:, :], rwF[:, :])
                    cw32 = ep.tile([128, 128], i32, tag="cw32")
                    nc.vector.tensor_copy(cw32[:, :], cwF[:, :])
                    dv_ap = lambda t_: t_.rearrange(
                        "(g e) -> g e", g=GPC
                    )[R * 8 + gq * 4:R * 8 + gq * 4 + 4, :].rearrange(
                        "g (q c f) -> g q c f", q=NQ, c=32
                    )[:, Qq, :, :]
                    nc.sync.dma_start(dv_ap(e_new[0, :]), rw32[:, :])
                    nc.sync.dma_start(dv_ap(e_new[1, :]), cw32[:, :])
                    nc.sync.dma_start(dv_ap(valid_out[:]), valdF[:, :])


            tabs = {}
            nm_rows = nm_dram[:, :].rearrange("a b -> (a b)").rearrange(
                "(g j) -> g j", g=GPC
            )
            for R in range(NROUND):
                tab = tbp.tile([128, N_PER], i32, tag="tab")
                for j in range(8):
                    nc.sync.dma_start(
                        tab[:, :][16 * j:16 * (j + 1), :],
                        nm_rows[R * 8 + j, :].partition_broadcast(16),
                    )
                tabs[R] = tab
            for R in range(NROUND):
                for Qq in range(NQ):
                    if (R, Qq) in DVE_PAIRS:
                        continue
                    evs = {}
                    for side in range(2):
                        og = ogp.tile([128, LIDX], i32, tag=f"og{side}")
                        nc.gpsimd.ap_gather(
                            og[:, :].rearrange("p (n d) -> p n d", d=1),
                            tabs[R][:, :].rearrange("p (n d) -> p n d", d=1),
                            l16s[(R, Qq, side)][:, :],
                            channels=128, num_elems=N_PER, d=1, num_idxs=LIDX,
                        )
                        # extract partition p's own e-slice og[p, w*LPP:
                        # (w+1)*LPP] (w = p%16) via a DRAM bounce: SBUF
                        # partition-stepped APs are unreliable; the DRAM-side
                        # diagonal is plain address math.
                        ogd = dp.tile([128, LIDX], i32, tag=f"ogd{side}")
                        nc.sync.dma_start(ogd[:, :], og[:, :])
                        ev = vp.tile([128, LPP], i32, tag="ev")
                        ogb = ogd[:, :]
                        diag = bass.AP(
                            tensor=ogb.tensor, offset=ogb.offset,
                            ap=[[16 * LIDX, 8], [LIDX + LPP, 16], [1, LPP]],
                        )
                        nc.gpsimd.dma_start(ev[:, :], diag)
                        evs[side] = ev
                    rv, cv = evs[0], evs[1]
                    am = ep.tile([128, LPP], i32, tag="am")
                    nc.vector.tensor_scalar(am[:, :], rv[:, :], 0, None, op0=Alu.is_ge)
                    bm = ep.tile([128, LPP], i32, tag="bm")
                    nc.vector.tensor_scalar(bm[:, :], cv[:, :], 0, None, op0=Alu.is_ge)
                    vald = ep.tile([128, LPP], u8, tag="vald")
                    nc.vector.tensor_tensor(vald[:, :], am[:, :], bm[:, :], op=Alu.mult)
                    ta = ep.tile([128, LPP], i32, tag="ta")
                    nc.vector.tensor_scalar(
                        ta[:, :], am[:, :], (1 << 20), 1, op0=Alu.mult, op1=Alu.subtract
                    )
                    tb = ep.tile([128, LPP], i32, tag="tb")
                    nc.vector.tensor_scalar(
                        tb[:, :], bm[:, :], (1 << 20), 1, op0=Alu.mult, op1=Alu.subtract
                    )
                    rw = ep.tile([128, LPP], i32, tag="rw")
                    nc.vector.tensor_tensor(rw[:, :], rv[:, :], tb[:, :], op=Alu.min)
                    cw = ep.tile([128, LPP], i32, tag="cw")
                    nc.vector.tensor_tensor(cw[:, :], cv[:, :], ta[:, :], op=Alu.min)
                    out_ap = lambda t_: t_.rearrange(
                        "(g e) -> g e", g=GPC
                    )[R * 8:(R + 1) * 8, :].rearrange(
                        "g (q w s) -> g q w s", q=NQ, w=16
                    )[:, Qq, :, :]
                    nc.sync.dma_start(out_ap(e_new[0, :]), rw[:, :])
                    nc.sync.dma_start(out_ap(e_new[1, :]), cw[:, :])
                    nc.sync.dma_start(out_ap(valid_out[:]), vald[:, :])


    nc.compile()
    if not nc.is_finalized():
        nc.finalize()
    return nc


def _get_nc():
    if "nc" not in _cache:
        _cache["nc"] = _build_nc()
    return _cache["nc"]


def _make_in_maps(x, edge_index, pool_vector):
    in_maps = []
    p = np.arange(128, dtype=np.int64)
    for c in range(NCORES):
        consts = np.zeros((128, 44), dtype=np.float32)
        consts[:, 0] = c * NPC
        consts[:, 1] = p * N_PER
        consts[:, 2] = c * KPC + p * K
        consts[:, 3] = c * GPC + p
        consts[:, 4] = c * NPC + p * N_PER
        for R in range(NROUND):
            consts[:, 5 + R] = (c * GPC + R * 8 + p // 16) * N_PER
        for gg in range(GPC):
            consts[:, 12 + gg] = (c * GPC + gg) * N_PER
        esh = edge_index[:, c * EPC:(c + 1) * EPC]
        # 16-way interleave per 4096-edge quarter so the device's wrapped
        # per-core ap_gather lists unwrap to plain edge order (fixed,
        # data-independent permutation).
        ew = np.ascontiguousarray(
            esh.reshape(2, GPC, NQ, LPP, 16).transpose(0, 1, 2, 4, 3)
        ).reshape(2, EPC)
        eic = np.ascontiguousarray(
            esh.reshape(2, GPC, NQ, 32, 128).transpose(0, 1, 2, 4, 3)
        ).reshape(2, EPC)
        in_maps.append({
            "x": np.ascontiguousarray(x[c * NPC:(c + 1) * NPC]),
            "edge_index": ew,
            "ei_cols": eic,
            "pool_vector": np.ascontiguousarray(
                pool_vector[c * GPC:(c + 1) * GPC]
            ),
            "consts": consts,
        })
    return in_maps


def kernel(x, edge_index, batch, pool_vector, c_size):
    import os
    from concourse.bass_utils import run_bass_kernel_spmd

    x = np.asarray(x, dtype=np.float32)
    edge_index = np.asarray(edge_index, dtype=np.int32)
    pool_vector = np.asarray(pool_vector, dtype=np.float32)

    nc = _get_nc()
    in_maps = _make_in_maps(x, edge_index, pool_vector)
    trace = bool(os.environ.get("KERNEL_TRACE"))
    res = run_bass_kernel_spmd(
        nc, in_maps, core_ids=list(range(NCORES)), trace=trace,
        tmpdir=os.environ.get("KERNEL_TRACE_DIR") or None,
    )
    if trace:
        _cache["last_exec_time_ns"] = res.exec_time_ns
        _cache["last_results_obj"] = res
    rs = res.results

    x_out = np.concatenate([r["x_out"] for r in rs], axis=0)
    edge_new = np.concatenate([r["edge_new"] for r in rs], axis=1)
    batch_o = np.concatenate([r["batch_out"] for r in rs], axis=0)
    perm = np.concatenate([r["perm"] for r in rs], axis=0)
    valid = np.concatenate([r["valid"] for r in rs], axis=0) != 0
    return x_out, edge_new.astype(np.int32), batch_o.astype(np.int32), \
        perm.astype(np.int32), valid


# revision 30
# speedup vs baseline: 1.2199x; 1.2199x over previous
"""CAGPool layer (score -> per-graph top-k -> gather/gate -> edge re-index)
as a Bass kernel on 8 Trainium2 NeuronCores.

Sharding: data-parallel by graph. Each of the 8 cores owns 32 consecutive
graphs (32*512 nodes, 32*16384 edges). No cross-core communication.

Per-core pipeline:
  1. scores d[i] = x[i] . pool_vector[graph(i)]  (DVE dot products, f32)
  2. per-graph ordered top-256 via DVE max8/max_index/match_replace rounds
  3. nodemap (local node id -> new global id + 1, 0 if dropped) via gpsimd
     local_scatter; affine fixup to (new id | -1) on DVE
  4. edge re-indexing: per-edge table lookups via gpsimd ap_gather
     (8 Q7 cores, one graph per core per round, 16-way wrapped lists),
     de-interleaved to edge order on the Scalar engine, masked on DVE
  5. x_out: 128-row indirect-DMA gathers (one offset per partition) from x,
     scaled by sigmoid(score/||p||) with per-partition scalars
"""

import numpy as np

NCORES = 8
B, N_PER, D, DEG = 256, 512, 256, 32
K = 256                      # kept nodes per graph (ceil(0.5 * 512))
GPC = B // NCORES            # 32 graphs per core
NPC = GPC * N_PER            # 16384 nodes per core
KPC = GPC * K                # 8192 kept nodes per core
EPG = N_PER * DEG            # 16384 edges per graph
EPC = GPC * EPG              # 524288 edges per core

NROUND = 4                   # graph rounds (8 graphs each, one per Q7 core)
NQ = 4                       # list quarters per round
LIDX = EPG // NQ             # 4096 indices per ap_gather call per core
LPP = LIDX // 16             # 256 list entries per partition

_cache = {}


def _build_nc():
    import concourse.bass as bass
    import concourse.bacc as bacc
    import concourse.mybir as mybir
    from concourse import tile
    from concourse.bass import IndirectOffsetOnAxis
    from concourse.masks import make_identity

    f32 = mybir.dt.float32
    i32 = mybir.dt.int32
    i16 = mybir.dt.int16
    u32 = mybir.dt.uint32
    u8 = mybir.dt.uint8
    Alu = mybir.AluOpType
    Act = mybir.ActivationFunctionType

    nc = bacc.Bacc("TRN2", target_bir_lowering=False, debug=False)

    x = nc.dram_tensor("x", [NPC, D], f32, kind="ExternalInput")
    ei = nc.dram_tensor("edge_index", [2, EPC], i32, kind="ExternalInput")
    pv = nc.dram_tensor("pool_vector", [GPC, D], f32, kind="ExternalInput")
    # per-core f32 constants, per partition row p:
    #   col0: core*NPC                col1: p*N_PER
    #   col2: core*KPC + p*K          col3: core*GPC + p
    #   col4: core*NPC + p*N_PER      col5+R: (core*GPC + R*8 + p//16)*N_PER
    consts = nc.dram_tensor("consts", [128, 12], f32, kind="ExternalInput")

    x_out = nc.dram_tensor("x_out", [KPC, D], f32, kind="ExternalOutput")
    e_new = nc.dram_tensor("edge_new", [2, EPC], i32, kind="ExternalOutput")
    batch_out = nc.dram_tensor("batch_out", [KPC], i32, kind="ExternalOutput")
    perm_out = nc.dram_tensor("perm", [KPC], i32, kind="ExternalOutput")
    valid_out = nc.dram_tensor("valid", [EPC], u8, kind="ExternalOutput")

    with tile.TileContext(nc) as tc:
        with (
            tc.tile_pool(name="persist", bufs=1) as pp,
            tc.tile_pool(name="xload", bufs=3) as xp,
            tc.tile_pool(name="edge", bufs=2) as ep,
            tc.tile_pool(name="tabs", bufs=4) as tbp,
            tc.tile_pool(name="og", bufs=2) as ogp,
            tc.tile_pool(name="l16s", bufs=32) as lp,
            tc.tile_pool(name="evs", bufs=6) as vp,
            tc.tile_pool(name="xg", bufs=8) as gp,
            tc.tile_pool(name="psum", bufs=2, space="PSUM") as psp,
            tc.tile_pool(name="dram", bufs=1, space="DRAM") as dp,
        ):
            # ---- persistent small tiles ----
            consts_sb = pp.tile([128, 12], f32, tag="consts")
            nc.sync.dma_start(consts_sb[:, :], consts[:, :])
            pv_sb = pp.tile([GPC, D], f32, tag="pv_sb")
            nc.sync.dma_start(pv_sb[:, :], pv[:, :])
            pv_bc = pp.tile([128, GPC * D], f32, tag="pv_bc")
            nc.sync.dma_start(
                pv_bc[:, :],
                pv[:, :].rearrange("a b -> (a b)").partition_broadcast(128),
            )
            ident = pp.tile([128, 128], f32, tag="ident")
            make_identity(nc, ident[:, :])

            gnode_off = consts_sb[0:GPC, 1:2]       # [32,1] p*N_PER
            gnew_off = consts_sb[0:GPC, 2:3]        # [32,1] core*KPC + p*K
            g_id = consts_sb[0:GPC, 3:4]            # [32,1] core*GPC + p
            gnode_glob = consts_sb[0:GPC, 4:5]      # [32,1] core*NPC + p*N_PER

            # ---- phase 1: scores (d = x . p_g) ----
            sc_raw = pp.tile([128, 128], f32, tag="sc_raw")
            junk = pp.tile([128, D], f32, tag="junk")
            TGROUP = 8
            for tg in range(128 // TGROUP):
                xt = xp.tile([128, TGROUP * D], f32, tag="xt")
                nc.sync.dma_start(
                    xt[:, :].rearrange("p (t d) -> p t d", t=TGROUP),
                    x[tg * TGROUP * 128:(tg + 1) * TGROUP * 128, :].rearrange(
                        "(t p) d -> p t d", p=128
                    ),
                )
                for t_in in range(TGROUP):
                    t = tg * TGROUP + t_in
                    g = t // 4
                    nc.vector.scalar_tensor_tensor(
                        out=junk[:, :],
                        in0=xt[:, t_in * D:(t_in + 1) * D],
                        scalar=0.0,
                        in1=pv_bc[:, g * D:(g + 1) * D],
                        op0=Alu.bypass,
                        op1=Alu.mult,
                        accum_out=sc_raw[:, t:t + 1],
                    )

            # ---- phase 2: transpose scores to [graph, node] layout ----
            psT = psp.tile([128, 128], f32, tag="psT")
            nc.tensor.transpose(psT[:, :], sc_raw[:, :], ident[:, :])
            scT = pp.tile([128, 128], f32, tag="scT")
            nc.scalar.copy(scT[:, :], psT[:, :])
            sc_rt = dp.tile([128, 128], f32, tag="sc_rt")
            nc.sync.dma_start(sc_rt[:, :], scT[:, :])
            work = pp.tile([GPC, N_PER], f32, tag="work")
            nc.sync.dma_start(
                work[:, :],
                sc_rt[:, :].rearrange("a b -> (a b)").rearrange("(g j) -> g j", g=GPC),
            )

            # ---- phase 3: ordered top-256 per graph ----
            vals = pp.tile([GPC, K], f32, tag="vals")
            idxs = pp.tile([GPC, K], u32, tag="idxs")
            NR = K // 8
            for r in range(NR):
                sl = slice(8 * r, 8 * (r + 1))
                nc.vector.max(vals[:, sl], work[:, :])
                nc.vector.max_index(idxs[:, sl], vals[:, sl], work[:, :])
                if r != NR - 1:
                    nc.vector.match_replace(
                        work[:, :], vals[:, sl], work[:, :], imm_value=-1e30
                    )

            # ---- phase 4: gate + perm + batch ----
            junk2 = pp.tile([GPC, D], f32, tag="junk2")
            nrm2 = pp.tile([GPC, 1], f32, tag="nrm2")
            nc.vector.scalar_tensor_tensor(
                out=junk2[:, :], in0=pv_sb[:, :], scalar=0.0, in1=pv_sb[:, :],
                op0=Alu.bypass, op1=Alu.mult, accum_out=nrm2[:, :],
            )
            nrm = pp.tile([GPC, 1], f32, tag="nrm")
            nc.scalar.activation(nrm[:, :], nrm2[:, :], Act.Sqrt)
            invn = pp.tile([GPC, 1], f32, tag="invn")
            nc.vector.reciprocal(invn[:, :], nrm[:, :])
            gate = pp.tile([GPC, K], f32, tag="gate")
            nc.scalar.activation(gate[:, :], vals[:, :], Act.Sigmoid, scale=invn[:, :])

            idx32 = pp.tile([GPC, K], i32, tag="idx32")
            nc.vector.tensor_copy(idx32[:, :], idxs[:, :])
            perm_t = pp.tile([GPC, K], i32, tag="perm_t")
            nc.vector.tensor_scalar(
                perm_t[:, :], idx32[:, :], gnode_glob, None, op0=Alu.add
            )
            nc.sync.dma_start(
                perm_out[:].rearrange("(g r) -> g r", g=GPC), perm_t[:, :]
            )
            batch_t = pp.tile([GPC, K], i32, tag="batch_t")
            nc.vector.memset(batch_t[:, :], 0)
            nc.vector.tensor_scalar(
                batch_t[:, :], batch_t[:, :], g_id, None, op0=Alu.add
            )
            nc.sync.dma_start(
                batch_out[:].rearrange("(g r) -> g r", g=GPC), batch_t[:, :]
            )

            # ---- phase 4b: j-ordered [128, 64] layouts of row ids and gates
            # (OFF[p, c] = local row id of x_out row j = c*128 + p; GT same for
            # gate).  j = g*256 + r -> p = r % 128, c = 2g + r//128, so OFF is
            # the PE transpose of idx rows split in two 128-halves.
            idxF = pp.tile([GPC, K], f32, tag="idxF")
            nc.vector.tensor_copy(idxF[:, :], idx32[:, :])
            nc.vector.tensor_scalar(
                idxF[:, :], idxF[:, :], gnode_off, None, op0=Alu.add
            )
            OFFf = pp.tile([128, 2 * GPC], f32, tag="OFFf")
            GTt = pp.tile([128, 2 * GPC], f32, tag="GTt")
            for h in range(2):
                psA = psp.tile([128, GPC], f32, tag="psA")
                nc.tensor.transpose(
                    psA[:, :], idxF[:, h * 128:(h + 1) * 128], ident[0:GPC, 0:GPC]
                )
                nc.scalar.copy(
                    OFFf[:, :].rearrange("p (c h2) -> p c h2", h2=2)[:, :, h],
                    psA[:, :],
                )
                psB = psp.tile([128, GPC], f32, tag="psB")
                nc.tensor.transpose(
                    psB[:, :], gate[:, h * 128:(h + 1) * 128], ident[0:GPC, 0:GPC]
                )
                nc.scalar.copy(
                    GTt[:, :].rearrange("p (c h2) -> p c h2", h2=2)[:, :, h],
                    psB[:, :],
                )
            OFF = pp.tile([128, 2 * GPC], i32, tag="OFF")
            nc.vector.tensor_copy(OFF[:, :], OFFf[:, :])

            # ---- phase 7: x_out row gather + sigmoid gating ----
            for c in range(KPC // 128):
                xg = gp.tile([128, D], f32, tag="xgt")
                nc.gpsimd.indirect_dma_start(
                    xg[:, :], None, x[:, :],
                    IndirectOffsetOnAxis(ap=OFF[:, c:c + 1], axis=0),
                )
                nc.vector.tensor_scalar(
                    xg[:, :], xg[:, :], GTt[:, c:c + 1], None, op0=Alu.mult
                )
                nc.sync.dma_start(
                    x_out[c * 128:(c + 1) * 128, :], xg[:, :]
                )


            # ---- phase 5: nodemap table (v+1 scheme, 0 = dropped) ----
            # nm[g, v] = sum_r [idx[g, r] == v] * (r+1), built with 512
            # compare-accumulate STT ops (one per node slot v); runs on DVE
            # and hides completely under the pool-bound edge phase.
            idxsF = pp.tile([GPC, K], f32, tag="idxsF")
            nc.vector.tensor_copy(idxsF[:, :], idxs[:, :])
            rr_i = pp.tile([GPC, K], i32, tag="rr_i")
            nc.gpsimd.iota(rr_i[:, :], pattern=[[1, K]], base=1, channel_multiplier=0)
            rr_bc = pp.tile([GPC, K], f32, tag="rr_bc")
            nc.vector.tensor_copy(rr_bc[:, :], rr_i[:, :])
            junk3 = pp.tile([GPC, K], f32, tag="junk3")
            nmF = pp.tile([GPC, N_PER], f32, tag="nmF")
            for v in range(N_PER):
                nc.vector.scalar_tensor_tensor(
                    out=junk3[:, :], in0=idxsF[:, :], scalar=float(v),
                    in1=rr_bc[:, :], op0=Alu.is_equal, op1=Alu.mult,
                    accum_out=nmF[:, v:v + 1],
                )
            nm32 = pp.tile([GPC, N_PER], i32, tag="nm32")
            nc.vector.tensor_copy(nm32[:, :], nmF[:, :])
            nmask = pp.tile([GPC, N_PER], i32, tag="nmask")
            nc.vector.tensor_scalar(nmask[:, :], nm32[:, :], 0, None, op0=Alu.is_gt)
            nm_t1 = pp.tile([GPC, N_PER], i32, tag="nm_t1")
            nc.vector.tensor_scalar(
                nm_t1[:, :], nm32[:, :], gnew_off, None, op0=Alu.add
            )
            nm_t2 = pp.tile([GPC, N_PER], i32, tag="nm_t2")
            nc.vector.tensor_tensor(nm_t2[:, :], nm_t1[:, :], nmask[:, :], op=Alu.mult)
            nm = pp.tile([GPC, N_PER], i32, tag="nm")
            nc.vector.tensor_scalar(nm[:, :], nm_t2[:, :], 1, None, op0=Alu.subtract)
            nm_dram = dp.tile([GPC, N_PER], i32, tag="nm_dram")
            nc.sync.dma_start(nm_dram[:, :], nm[:, :])

            # ---- phase 6: edge re-indexing via ap_gather rounds ----
            # Round R: Q7 core j handles graph G = R*8 + j; its 16 partitions
            # hold graph G's nodemap (replicated).  The host passes the edge
            # array 16-way pre-interleaved per 4096-edge quarter, so the
            # wrapped per-core index list unwraps to plain edge order: the
            # gather output og[p, k] = nodemap[src[e = Qq*4096 + k]].  Each
            # partition then keeps its own 256-edge slice via one diagonal
            # SBUF->SBUF DMA and the mask/write phase runs on dense tiles.
            # 6a: all index lists first (keeps the in-order DVE stream from
            # blocking the pool's gathers behind later mask work)
            iters = [(R, Qq, side)
                     for R in range(NROUND) for Qq in range(NQ) for side in range(2)]
            l16s = {}
            for R, Qq, side in iters:
                eb = ep.tile([128, LPP], i32, tag="eb")
                nc.sync.dma_start(
                    eb[:, :],
                    ei[side, :]
                    .rearrange("(g e) -> g e", g=GPC)[R * 8:(R + 1) * 8, :]
                    .rearrange("g (q w s) -> g q w s", q=NQ, w=16)
                    [:, Qq, :, :],
                )
                loc = ep.tile([128, LPP], i32, tag="loc")
                nc.vector.tensor_scalar(
                    loc[:, :], eb[:, :], consts_sb[:, 5 + R:6 + R], None,
                    op0=Alu.subtract,
                )
                l16 = lp.tile([128, LPP], i16, tag="l16")
                nc.vector.tensor_copy(l16[:, :], loc[:, :])
                l16s[(R, Qq, side)] = l16

            # 6b: per (R, Qq): gathers + bounce extraction + masks + writes.
            # All DVE index prep already queued (6a), so the in-order DVE
            # stream can park on masks without starving the pool.
            tabs = {}
            nm_rows = nm_dram[:, :].rearrange("a b -> (a b)").rearrange(
                "(g j) -> g j", g=GPC
            )
            for R in range(NROUND):
                tab = tbp.tile([128, N_PER], i32, tag="tab")
                for j in range(8):
                    nc.sync.dma_start(
                        tab[:, :][16 * j:16 * (j + 1), :],
                        nm_rows[R * 8 + j, :].partition_broadcast(16),
                    )
                tabs[R] = tab
            for R in range(NROUND):
                for Qq in range(NQ):
                    evs = {}
                    for side in range(2):
                        og = ogp.tile([128, LIDX], i32, tag=f"og{side}")
                        nc.gpsimd.ap_gather(
                            og[:, :].rearrange("p (n d) -> p n d", d=1),
                            tabs[R][:, :].rearrange("p (n d) -> p n d", d=1),
                            l16s[(R, Qq, side)][:, :],
                            channels=128, num_elems=N_PER, d=1, num_idxs=LIDX,
                        )
                        # extract partition p's own e-slice og[p, w*LPP:
                        # (w+1)*LPP] (w = p%16) via a DRAM bounce: SBUF
                        # partition-stepped APs are unreliable; the DRAM-side
                        # diagonal is plain address math.
                        ogd = dp.tile([128, LIDX], i32, tag=f"ogd{side}")
                        nc.sync.dma_start(ogd[:, :], og[:, :])
                        ev = vp.tile([128, LPP], i32, tag="ev")
                        ogb = ogd[:, :]
                        diag = bass.AP(
                            tensor=ogb.tensor, offset=ogb.offset,
                            ap=[[16 * LIDX, 8], [LIDX + LPP, 16], [1, LPP]],
                        )
                        nc.gpsimd.dma_start(ev[:, :], diag)
                        evs[side] = ev
                    rv, cv = evs[0], evs[1]
                    am = ep.tile([128, LPP], i32, tag="am")
                    nc.vector.tensor_scalar(am[:, :], rv[:, :], 0, None, op0=Alu.is_ge)
                    bm = ep.tile([128, LPP], i32, tag="bm")
                    nc.vector.tensor_scalar(bm[:, :], cv[:, :], 0, None, op0=Alu.is_ge)
                    vald = ep.tile([128, LPP], u8, tag="vald")
                    nc.vector.tensor_tensor(vald[:, :], am[:, :], bm[:, :], op=Alu.mult)
                    ta = ep.tile([128, LPP], i32, tag="ta")
                    nc.vector.tensor_scalar(
                        ta[:, :], am[:, :], (1 << 20), 1, op0=Alu.mult, op1=Alu.subtract
                    )
                    tb = ep.tile([128, LPP], i32, tag="tb")
                    nc.vector.tensor_scalar(
                        tb[:, :], bm[:, :], (1 << 20), 1, op0=Alu.mult, op1=Alu.subtract
                    )
                    rw = ep.tile([128, LPP], i32, tag="rw")
                    nc.vector.tensor_tensor(rw[:, :], rv[:, :], tb[:, :], op=Alu.min)
                    cw = ep.tile([128, LPP], i32, tag="cw")
                    nc.vector.tensor_tensor(cw[:, :], cv[:, :], ta[:, :], op=Alu.min)
                    out_ap = lambda t_: t_.rearrange(
                        "(g e) -> g e", g=GPC
                    )[R * 8:(R + 1) * 8, :].rearrange(
                        "g (q w s) -> g q w s", q=NQ, w=16
                    )[:, Qq, :, :]
                    nc.sync.dma_start(out_ap(e_new[0, :]), rw[:, :])
                    nc.sync.dma_start(out_ap(e_new[1, :]), cw[:, :])
                    nc.sync.dma_start(out_ap(valid_out[:]), vald[:, :])

    nc.compile()
    if not nc.is_finalized():
        nc.finalize()
    return nc


def _get_nc():
    if "nc" not in _cache:
        _cache["nc"] = _build_nc()
    return _cache["nc"]


def _make_in_maps(x, edge_index, pool_vector):
    in_maps = []
    p = np.arange(128, dtype=np.int64)
    for c in range(NCORES):
        consts = np.zeros((128, 12), dtype=np.float32)
        consts[:, 0] = c * NPC
        consts[:, 1] = p * N_PER
        consts[:, 2] = c * KPC + p * K
        consts[:, 3] = c * GPC + p
        consts[:, 4] = c * NPC + p * N_PER
        for R in range(NROUND):
            consts[:, 5 + R] = (c * GPC + R * 8 + p // 16) * N_PER
        esh = edge_index[:, c * EPC:(c + 1) * EPC]
        # 16-way interleave per 4096-edge quarter so the device's wrapped
        # per-core ap_gather lists unwrap to plain edge order (fixed,
        # data-independent permutation).
        ew = np.ascontiguousarray(
            esh.reshape(2, GPC, NQ, LPP, 16).transpose(0, 1, 2, 4, 3)
        ).reshape(2, EPC)
        in_maps.append({
            "x": np.ascontiguousarray(x[c * NPC:(c + 1) * NPC]),
            "edge_index": ew,
            "pool_vector": np.ascontiguousarray(
                pool_vector[c * GPC:(c + 1) * GPC]
            ),
            "consts": consts,
        })
    return in_maps


def kernel(x, edge_index, batch, pool_vector, c_size):
    import os
    from concourse.bass_utils import run_bass_kernel_spmd

    x = np.asarray(x, dtype=np.float32)
    edge_index = np.asarray(edge_index, dtype=np.int32)
    pool_vector = np.asarray(pool_vector, dtype=np.float32)

    nc = _get_nc()
    in_maps = _make_in_maps(x, edge_index, pool_vector)
    trace = bool(os.environ.get("KERNEL_TRACE"))
    res = run_bass_kernel_spmd(
        nc, in_maps, core_ids=list(range(NCORES)), trace=trace,
        tmpdir=os.environ.get("KERNEL_TRACE_DIR") or None,
    )
    if trace:
        _cache["last_exec_time_ns"] = res.exec_time_ns
        _cache["last_results_obj"] = res
    rs = res.results

    x_out = np.concatenate([r["x_out"] for r in rs], axis=0)
    edge_new = np.concatenate([r["edge_new"] for r in rs], axis=1)
    batch_o = np.concatenate([r["batch_out"] for r in rs], axis=0)
    perm = np.concatenate([r["perm"] for r in rs], axis=0)
    valid = np.concatenate([r["valid"] for r in rs], axis=0) != 0
    return x_out, edge_new.astype(np.int32), batch_o.astype(np.int32), \
        perm.astype(np.int32), valid
